# revision 9
# baseline (speedup 1.0000x reference)
"""CSA Lightning Indexer on 8 Trainium2 NeuronCores (Bass/Tile).

Reference computation (per batch b):
    qi = q[b] @ Wq.T            # [Lq, 2048] viewed as [Lq, H=4, Dh=512]
    ki = k[b] @ Wk.T            # [Lc, 2048] viewed as [Lc, 4, 512]
    w  = q[b] @ Wg.T            # [Lq, 4]
    scores[b,i,k] = sum_h relu(qi[i,h]·ki[k,h]) * w[i,h]

Sharding: (B=2, Lq=4096) flattened to 8192 query rows, 1024 rows per core
(cores 0-3 batch 0, cores 4-7 batch 1). The ki projection (shared by the 4
cores of a batch group) is sharded 4-way over its output dim: each core
computes 4 of the 16 j-tiles, the group AllGathers the full kiT through
DRAM bounce buffers while qproj runs, then every core reads the complete
[2048, 1024] kiT back into SBUF for the qk stage. This cuts per-core PE
work from ~298us to ~216us vs computing ki redundantly per core.

Matmuls run in float16 (fp8 was measured at 3.6e-2..6.4e-2 rel err vs the
2e-2 gate in every placement, so DoubleRow fp8 is not usable) with fp32
PSUM accumulation; the tiny gate-vector path stays float32r.

Scheduling notes (from the perfetto/NTFF profile of this kernel):
  - The PE executes one [128,512] fp16 matmul per 263ns flat from the first
    matmul to the last (~1.95GHz mid p-state; the collective machinery pins
    the clock there for the whole run, including matmuls issued before the
    CC dispatch). PE busy time ~245us is 94% of the kernel, so everything
    else is about keeping the PE fed from instruction 1 and cutting the
    head/tail around the matmul stream.
  - One dma_start costs ~600ns of ISSUE time on the issuing engine's queue,
    regardless of transfer size; the hardware queue itself fans out over 16
    DMA engines at 300-400GB/s aggregate. The input ramp is issue-rate
    bound, not bandwidth bound => inputs are grouped into ~1MB issues and
    split across BOTH hardware DGE queues (sync/SP and scalar/Activation):
    sync carries kT + wq + staging/readback/outputs, scalar carries
    wk + qT + wg/eye. kproj-critical pieces are issued first in small
    chunks so the first matmul starts ~10us in.
  - The 4-rank 1MB-per-rank AllGather drains at ~51GB/s (~61us) and is
    gated behind the wq-pair-4 prefetch (dispatch ~55-60us); it completes
    ~120-140us, well before qk needs kiT (~190us).
  - Tail: the last (it,kh) unit is computed in two 256-column chunks whose
    epilogues + output DMAs go out on both queues, so the time from the
    last matmul to the last byte is ~3us instead of ~7us.
"""

from contextlib import ExitStack

import numpy as np

import concourse.bacc as bacc
import concourse.mybir as mybir
from concourse import tile
from concourse.bass_utils import run_bass_kernel_spmd

N_CORES = 8
B, LQ, LC, D, H, DH = 2, 4096, 1024, 2048, 4, 512
LQC = (B * LQ) // N_CORES  # 1024 query rows per core
ND = D // 128  # 16 d-tiles (contraction)
NG = ND // 4  # 4 d-groups (DMA granularity)
NJ = D // 128  # 16 j-tiles (projection output)
NJQ = NJ // 4  # 4 j-tiles computed locally for ki (quarter)
NKH = LC // 512  # 2 k halves
NIT = LQC // 128  # 8 i-tiles

F32 = mybir.dt.float32
R = mybir.dt.float16
RW = mybir.dt.float32r  # gate-vector path stays tf32-precision

REPLICA_GROUPS = [[0, 1, 2, 3], [4, 5, 6, 7]]

_CACHE = {}


def _build():
    nc = bacc.Bacc(num_devices=N_CORES)

    # d-grouped layouts: [g, p, dtl, cols] with d = g*512 + dtl*128 + p
    qT = nc.dram_tensor("qT", [NG, 128, 4, LQC], R, kind="ExternalInput")
    kT = nc.dram_tensor("kT", [NG, 128, 4, LC], R, kind="ExternalInput")
    # wq pairs: [t, p, tl, dt*128+j] holds j-tiles (2t, 2t+1)
    wq2 = nc.dram_tensor("wq2", [NJ // 2, 128, 2, D], R, kind="ExternalInput")
    wk2q = nc.dram_tensor("wk2q", [NJQ, 128, D], R, kind="ExternalInput")
    wg2 = nc.dram_tensor("wg2", [128, ND * H], R, kind="ExternalInput")
    eye4 = nc.dram_tensor("eye4", [4, 4], RW, kind="ExternalInput")
    scores = nc.dram_tensor("scores", [LQC, LC], F32, kind="ExternalOutput")

    with tile.TileContext(nc) as tc:
        with (
            tc.tile_pool(name="kiT", bufs=1) as kiT_pool,
            tc.tile_pool(name="misc", bufs=1) as misc_pool,
            tc.tile_pool(name="dve", bufs=1) as dve_pool,
            tc.tile_pool(name="qT", bufs=1) as qT_pool,
            tc.tile_pool(name="wqblk", bufs=5) as wqblk_pool,
            tc.tile_pool(name="agdram", bufs=1, space="DRAM") as ag_pool,
        ):
            # rank m's 1MB chunk is p-major [128, (jl*2+kh)*512+k]; the
            # AllGather concatenates rank chunks, so ag_out[m] is rank m's 4
            # j-tiles (global j-tile 4m+jl) in exactly the kiT tile layout.
            ag_in = ag_pool.tile([128, NJQ * NKH * 512], R, name="ag_in")
            ag_out = ag_pool.tile([4, 128, NJQ * NKH * 512], R, name="ag_out")

            # kiT per source rank m: [p, jl, kh*512+k]; head h == rank h's
            # tile (head h spans global j-tiles 4h..4h+3).
            kiT = [
                kiT_pool.tile([128, 4, LC], R, tag=f"kiT{m}", name=f"kiT{m}")
                for m in range(4)
            ]

            # The input ramp is paced by the DMA fabric warming up
            # (~135->300GB/s over the first 15us), so the kproj schedule is
            # shaped around supply: dt-OUTER chains over two jl at a time
            # consume each landing 256KB kT piece with 4 matmuls (~243GB/s
            # demand) instead of bursting through kT at 2 matmuls per piece
            # (~486GB/s demand, guaranteed starvation). kT pieces alternate
            # between the two HW DGE queues so they ride the full aggregate
            # ramp in landing order.
            # PSUM pools for stages A+B-proj share one lifetime (4+2+1+1
            # banks) so no pool-close sync lands between kproj, wproj and
            # qproj; they close together right before qk claims all 8 banks.
            ps_stack = ExitStack()
            psA_pool = ps_stack.enter_context(
                tc.tile_pool(name="psA", bufs=4, space="PSUM")
            )
            psB_pool = ps_stack.enter_context(
                tc.tile_pool(name="psB", bufs=2, space="PSUM")
            )
            psw_pool = ps_stack.enter_context(
                tc.tile_pool(name="psw", bufs=1, space="PSUM")
            )
            with (
                nc.named_scope("kproj"),
                tc.tile_pool(name="kT", bufs=1) as kT_pool,
                tc.tile_pool(name="wkblk", bufs=1) as wkblk_pool,
                tc.tile_pool(name="kstg", bufs=8) as kstg_pool,
            ):
                wkb_sb = [
                    wkblk_pool.tile([128, D], R, tag=f"wkb{jl}", name=f"wkb{jl}")
                    for jl in range(NJQ)
                ]
                kT_sb = [
                    kT_pool.tile([128, 4, LC], R, tag=f"kTg{g}", name=f"kTg{g}")
                    for g in range(NG)
                ]
                # phase-1 weights (jl 0/1, d-halves) lead on scalar
                for jl in range(2):
                    nc.scalar.dma_start(
                        out=wkb_sb[jl][:, 0:1024], in_=wk2q[jl][:, 0:1024]
                    )
                # kT per-dtile pieces: even global dt on sync, odd on scalar
                for dt in range(ND):
                    g, dtl = dt // 4, dt % 4
                    eng = nc.sync if dt % 2 == 0 else nc.scalar
                    eng.dma_start(
                        out=kT_sb[g][:, dtl : dtl + 1, :],
                        in_=kT[g, :, dtl : dtl + 1, :],
                    )
                for jl in range(2):
                    nc.scalar.dma_start(
                        out=wkb_sb[jl][:, 1024:2048], in_=wk2q[jl][:, 1024:2048]
                    )
                for jl in range(2, NJQ):
                    nc.scalar.dma_start(out=wkb_sb[jl][:], in_=wk2q[jl])
                # stage-B inputs stream behind stage A's on both queues.
                qT_sb = [
                    qT_pool.tile([128, 4, LQC], R, tag=f"qTg{g}", name=f"qTg{g}")
                    for g in range(NG)
                ]
                for g in range(NG):
                    nc.scalar.dma_start(out=qT_sb[g][:], in_=qT[g])
                wg_sb = misc_pool.tile([128, ND * H], R, tag="wg", name="wg_sb")
                nc.scalar.dma_start(out=wg_sb[:], in_=wg2[:])
                eye_sb = misc_pool.tile([4, 4], RW, tag="eye", name="eye_sb")
                nc.scalar.dma_start(out=eye_sb[:], in_=eye4[:])
                wqp = []
                for t in range(5):
                    w = wqblk_pool.tile(
                        [128, 2, D], R, tag="wqb", name=f"wqp{t}", bufs=5
                    )
                    nc.sync.dma_start(out=w[:], in_=wq2[t])
                    wqp.append(w)
                # two dt-outer phases of 4 concurrent chains each
                for ph in range(2):
                    jls = (0, 1) if ph == 0 else (2, 3)
                    ps = {
                        (jl, kh): psA_pool.tile(
                            [128, 512], F32, tag="psA", name=f"psA{jl}_{kh}"
                        )
                        for jl in jls
                        for kh in range(NKH)
                    }
                    for dt in range(ND):
                        g, dtl = dt // 4, dt % 4
                        for jl in jls:
                            for kh in range(NKH):
                                nc.tensor.matmul(
                                    ps[(jl, kh)][:],
                                    wkb_sb[jl][:, dt * 128 : (dt + 1) * 128],
                                    kT_sb[g][:, dtl : dtl + 1, kh * 512 : (kh + 1) * 512],
                                    start=(dt == 0),
                                    stop=(dt == ND - 1),
                                )
                    for jl in jls:
                        for kh in range(NKH):
                            stg = kstg_pool.tile(
                                [128, 512], R, tag="kstg", name=f"kstg{jl}_{kh}"
                            )
                            nc.scalar.copy(stg[:], ps[(jl, kh)][:])
                            c = jl * NKH + kh
                            nc.sync.dma_start(
                                out=ag_in[:, c * 512 : (c + 1) * 512], in_=stg[:]
                            )

                # CC dispatch held behind the wq pair-4 prefetch: the gather
                # (dispatch + ~12us trigger + ~50us drain) still completes
                # ~50us before qk needs kiT.
                delay_t = misc_pool.tile([128, 8], R, tag="ccdel", name="ccdel")
                nc.gpsimd.tensor_copy(delay_t[:], wqp[4][:, 1:2, 0:8])
                nc.gpsimd.collective_compute(
                    "AllGather",
                    mybir.AluOpType.bypass,
                    replica_groups=REPLICA_GROUPS,
                    ins=[ag_in.opt()],
                    outs=[ag_out.opt()],
                )

            # ---------------- stage B: full i range ---------------------
            with (
                tc.tile_pool(name="qiT", bufs=1) as qiT_pool,
                tc.tile_pool(name="wsb", bufs=1) as w_pool,
                tc.tile_pool(name="sc", bufs=3) as sc_pool,
                tc.tile_pool(name="tm", bufs=4) as tm_pool,
            ):
                if True:
                    # gate vector: wT[h, i] halves, then 4x128 PE transposes
                    with nc.named_scope("wproj"):
                        w4 = dve_pool.tile([4, LQC], RW, tag="w4", name="w4")
                        for ih in range(2):
                            psw = psw_pool.tile([4, 512], F32, tag="psw", name=f"psw{ih}")
                            for dt in range(ND):
                                g, dtl = dt // 4, dt % 4
                                nc.tensor.matmul(
                                    psw[:],
                                    wg_sb[:, dt * H : (dt + 1) * H],
                                    qT_sb[g][:, dtl : dtl + 1, ih * 512 : (ih + 1) * 512],
                                    start=(dt == 0),
                                    stop=(dt == ND - 1),
                                )
                            nc.vector.tensor_copy(w4[:, ih * 512 : (ih + 1) * 512], psw[:])
                        w_sb = []
                        for it in range(NIT):
                            pswt = psw_pool.tile([128, 4], F32, tag="pswt", name=f"pswt{it}")
                            nc.tensor.matmul(
                                pswt[:],
                                w4[:, it * 128 : (it + 1) * 128],
                                eye_sb[:],
                                start=True,
                                stop=True,
                            )
                            wt = w_pool.tile([128, 4], F32, tag=f"w{it}", name=f"w{it}")
                            nc.vector.tensor_copy(wt[:], pswt[:])
                            w_sb.append(wt)

                    # qiT = Wq-blocks.T @ qT (each weight block used once)
                    with nc.named_scope("qproj"):
                        qiT = []
                        for jt in range(NJ):
                            t, tl = jt // 2, jt % 2
                            if tl == 0 and 5 <= t + 2 <= 7:
                                w = wqblk_pool.tile(
                                    [128, 2, D], R, tag="wqb", name=f"wqp{t + 2}", bufs=5
                                )
                                nc.sync.dma_start(out=w[:], in_=wq2[t + 2])
                                wqp.append(w)
                            qi = qiT_pool.tile([128, LQC], R, tag=f"qiT{jt}", name=f"qiT{jt}")
                            for ih in range(2):
                                ps = psB_pool.tile([128, 512], F32, tag="psB", name=f"psB{jt}_{ih}")
                                for dt in range(ND):
                                    g, dtl = dt // 4, dt % 4
                                    nc.tensor.matmul(
                                        ps[:],
                                        wqp[t][:, tl : tl + 1, dt * 128 : (dt + 1) * 128],
                                        qT_sb[g][:, dtl : dtl + 1, ih * 512 : (ih + 1) * 512],
                                        start=(dt == 0),
                                        stop=(dt == ND - 1),
                                    )
                                nc.scalar.copy(qi[:, ih * 512 : (ih + 1) * 512], ps[:])
                            qiT.append(qi)

                    # full kiT comes back from the gather: one ~1MB DMA per
                    # source rank, landing in the [p, jl, kh*512+k] layout
                    # qk consumes directly.
                    for m in range(4):
                        nc.sync.dma_start(out=kiT[m][:], in_=ag_out[m : m + 1])

                ps_stack.close()  # free psA/psB/psw banks for qk's 8

                # qk + fused relu*w epilogue
                with (
                    nc.named_scope("qk"),
                    tc.tile_pool(name="psq", bufs=8, space="PSUM") as psq_pool,
                ):
                    for it in range(NIT):
                        sc = sc_pool.tile([128, LC], F32, tag="sc", name=f"sc{it}")
                        for kh in range(NKH):
                            last_unit = it == NIT - 1 and kh == NKH - 1
                            # the final unit runs in four 128-col chunks so
                            # the post-last-matmul epilogue + DMA is short
                            nch = 4 if last_unit else 1
                            cw = 512 // nch
                            for c in range(nch):
                                base = kh * 512 + c * cw
                                scs = sc[:, base : base + cw]
                                for h in range(H):
                                    psq = psq_pool.tile(
                                        [128, cw], F32, tag="psq",
                                        name=f"psq{it}_{kh}_{c}_{h}",
                                    )
                                    for j in range(4):
                                        nc.tensor.matmul(
                                            psq[:],
                                            qiT[h * 4 + j][:, it * 128 : (it + 1) * 128],
                                            kiT[h][:, j : j + 1, base : base + cw],
                                            start=(j == 0),
                                            stop=(j == 3),
                                        )
                                    if h == 0:
                                        nc.vector.tensor_scalar(
                                            out=scs,
                                            in0=psq[:],
                                            scalar1=0.0,
                                            scalar2=w_sb[it][:, 0:1],
                                            op0=mybir.AluOpType.max,
                                            op1=mybir.AluOpType.mult,
                                        )
                                    else:
                                        tm = tm_pool.tile(
                                            [128, cw], F32, tag="tm",
                                            name=f"tm{it}_{kh}_{c}_{h}",
                                        )
                                        nc.scalar.activation(
                                            tm[:], psq[:], mybir.ActivationFunctionType.Relu
                                        )
                                        nc.vector.scalar_tensor_tensor(
                                            out=scs,
                                            in0=tm[:],
                                            scalar=w_sb[it][:, h : h + 1],
                                            in1=scs,
                                            op0=mybir.AluOpType.mult,
                                            op1=mybir.AluOpType.add,
                                        )
                                dst = scores[
                                    it * 128 : (it + 1) * 128, base : base + cw
                                ]
                                if last_unit and c % 2 == 1:
                                    # the issuing engine (scalar) just ran
                                    # this chunk's relu; the issue slots in
                                    # while the DVE finishes the stt
                                    nc.scalar.dma_start(out=dst, in_=scs)
                                else:
                                    nc.sync.dma_start(out=dst, in_=scs)
    nc.finalize()
    return nc


def _get_program():
    if "nc" not in _CACHE:
        _CACHE["nc"] = _build()
    return _CACHE["nc"]


def _tile_weight(w):
    # [j, d] nn.Linear weight -> [jt, p, dt*128+jcol] blocks where
    # block[jt][p, dt*128+j] = W.T[dt*128+p, jt*128+j]
    a = w.reshape(NJ, 128, ND, 128)  # [jt, j, dt, p]
    return np.ascontiguousarray(a.transpose(0, 3, 2, 1)).reshape(NJ, 128, D)


def _group_dmajor(xT):
    # [D, cols] d-major -> [g, p, dtl, cols] with d = g*512 + dtl*128 + p
    cols = xT.shape[1]
    a = xT.reshape(NG, 4, 128, cols)  # [g, dtl, p, cols]
    return np.ascontiguousarray(a.transpose(0, 2, 1, 3))


def _shard_inputs(q, k_compressed, Wq, Wk, Wg):
    ndt = np.float16
    wq_blocks = _tile_weight(np.asarray(Wq, dtype=np.float32)).astype(ndt)
    # pair j-tiles (2t, 2t+1): [t, p, tl, d]
    wq2 = np.ascontiguousarray(
        wq_blocks.reshape(NJ // 2, 2, 128, D).transpose(0, 2, 1, 3)
    )
    wk2 = _tile_weight(np.asarray(Wk, dtype=np.float32)).astype(ndt)
    # wg2[p, dt*4+h] = Wg.T[dt*128+p, h]
    g = np.asarray(Wg, dtype=np.float32).reshape(H, ND, 128)  # [h, dt, p]
    wg2 = np.ascontiguousarray(g.transpose(2, 1, 0)).reshape(128, ND * H).astype(ndt)
    eye = np.eye(4, dtype=np.float32)

    in_maps = []
    for c in range(N_CORES):
        b = c // (N_CORES // B)
        cq = c % (N_CORES // B)
        i0 = cq * LQC
        qTc = _group_dmajor(
            np.ascontiguousarray(
                np.asarray(q[b, i0 : i0 + LQC, :], dtype=np.float32).T
            ).astype(ndt)
        )
        kTc = _group_dmajor(
            np.ascontiguousarray(
                np.asarray(k_compressed[b], dtype=np.float32).T
            ).astype(ndt)
        )
        wk2q = np.ascontiguousarray(wk2[cq * NJQ : (cq + 1) * NJQ])
        in_maps.append(
            {"qT": qTc, "kT": kTc, "wq2": wq2, "wk2q": wk2q, "wg2": wg2, "eye4": eye}
        )
    return in_maps


def _run(inputs, trace=False, **kw):
    nc = _get_program()
    in_maps = _shard_inputs(**inputs)
    res = run_bass_kernel_spmd(nc, in_maps, list(range(N_CORES)), trace=trace, **kw)
    out = np.empty((B, LQ, LC), dtype=np.float32)
    for c in range(N_CORES):
        b = c // (N_CORES // B)
        i0 = (c % (N_CORES // B)) * LQC
        out[b, i0 : i0 + LQC, :] = res.results[c]["scores"]
    return out, res


def kernel(**inputs) -> np.ndarray:
    out, _ = _run(inputs)
    return out


# revision 14
# speedup vs baseline: 1.0772x; 1.0772x over previous
"""CSA Lightning Indexer on 8 Trainium2 NeuronCores (Bass/Tile).

Reference computation (per batch b):
    qi = q[b] @ Wq.T            # [Lq, 2048] viewed as [Lq, H=4, Dh=512]
    ki = k[b] @ Wk.T            # [Lc, 2048] viewed as [Lc, 4, 512]
    w  = q[b] @ Wg.T            # [Lq, 4]
    scores[b,i,k] = sum_h relu(qi[i,h]·ki[k,h]) * w[i,h]

Sharding: (B=2, Lq=4096) flattened to 8192 query rows, 1024 rows per core
(cores 0-3 batch 0, cores 4-7 batch 1). The ki projection (shared by the 4
cores of a batch group) is sharded 4-way over its output dim: each core
computes 4 of the 16 j-tiles, the group AllGathers the full kiT through
DRAM bounce buffers while qproj runs, then every core reads the complete
[2048, 1024] kiT back into SBUF for the qk stage. This cuts per-core PE
work from ~298us to ~216us vs computing ki redundantly per core.

Matmuls run in float16 (fp8 was measured at 3.6e-2..6.4e-2 rel err vs the
2e-2 gate in every placement, so DoubleRow fp8 is not usable) with fp32
PSUM accumulation; the tiny gate-vector path stays float32r.

Scheduling notes (from the perfetto/NTFF profile of this kernel):
  - The PE executes one [128,512] fp16 matmul per 263ns flat from the first
    matmul to the last (~1.95GHz mid p-state; the collective machinery pins
    the clock there for the whole run, including matmuls issued before the
    CC dispatch). PE busy time ~245us is 94% of the kernel, so everything
    else is about keeping the PE fed from instruction 1 and cutting the
    head/tail around the matmul stream.
  - One dma_start costs ~600ns of ISSUE time on the issuing engine's queue,
    regardless of transfer size; the hardware queue itself fans out over 16
    DMA engines at 300-400GB/s aggregate. The input ramp is issue-rate
    bound, not bandwidth bound => inputs are grouped into ~1MB issues and
    split across BOTH hardware DGE queues (sync/SP and scalar/Activation):
    sync carries kT + wq + staging/readback/outputs, scalar carries
    wk + qT + wg/eye. kproj-critical pieces are issued first in small
    chunks so the first matmul starts ~10us in.
  - The 4-rank 1MB-per-rank AllGather drains at ~51GB/s (~61us) and is
    gated behind the wq-pair-4 prefetch (dispatch ~55-60us); it completes
    ~120-140us, well before qk needs kiT (~190us).
  - Tail: the last (it,kh) unit is computed in two 256-column chunks whose
    epilogues + output DMAs go out on both queues, so the time from the
    last matmul to the last byte is ~3us instead of ~7us.
"""

from contextlib import ExitStack

import numpy as np

import concourse.bacc as bacc
import concourse.mybir as mybir
from concourse import tile
from concourse.bass_utils import run_bass_kernel_spmd

N_CORES = 8
B, LQ, LC, D, H, DH = 2, 4096, 1024, 2048, 4, 512
LQC = (B * LQ) // N_CORES  # 1024 query rows per core
ND = D // 128  # 16 d-tiles (contraction)
NG = ND // 4  # 4 d-groups (DMA granularity)
NJ = D // 128  # 16 j-tiles (projection output)
NJQ = NJ // 4  # 4 j-tiles computed locally for ki (quarter)
NKH = LC // 512  # 2 k halves
NIT = LQC // 128  # 8 i-tiles

F32 = mybir.dt.float32
R = mybir.dt.float16
RW = mybir.dt.float32r  # gate-vector path stays tf32-precision

REPLICA_GROUPS = [[0, 1, 2, 3], [4, 5, 6, 7]]

_CACHE = {}


def _build():
    nc = bacc.Bacc(num_devices=N_CORES)

    # d-grouped layouts: [g, p, dtl, cols] with d = g*512 + dtl*128 + p
    qT = nc.dram_tensor("qT", [NG, 128, 4, LQC], R, kind="ExternalInput")
    kT = nc.dram_tensor("kT", [NG, 128, 4, LC], R, kind="ExternalInput")
    # wq pairs: [t, p, tl, dt*128+j] holds j-tiles (2t, 2t+1)
    wq2 = nc.dram_tensor("wq2", [NJ // 2, 128, 2, D], R, kind="ExternalInput")
    wk2q = nc.dram_tensor("wk2q", [NJQ, 128, D], R, kind="ExternalInput")
    wg2 = nc.dram_tensor("wg2", [128, ND * H], R, kind="ExternalInput")
    eye4 = nc.dram_tensor("eye4", [4, 4], RW, kind="ExternalInput")
    scores = nc.dram_tensor("scores", [LQC, LC], F32, kind="ExternalOutput")

    with tile.TileContext(nc) as tc:
        with (
            tc.tile_pool(name="kiT", bufs=1) as kiT_pool,
            tc.tile_pool(name="misc", bufs=1) as misc_pool,
            tc.tile_pool(name="dve", bufs=1) as dve_pool,
            tc.tile_pool(name="qT", bufs=1) as qT_pool,
            tc.tile_pool(name="wqblk", bufs=5) as wqblk_pool,
            tc.tile_pool(name="agdram", bufs=1, space="DRAM") as ag_pool,
        ):
            # rank m's 1MB chunk is p-major [128, (jl*2+kh)*512+k]; the
            # AllGather concatenates rank chunks, so ag_out[m] is rank m's 4
            # j-tiles (global j-tile 4m+jl) in exactly the kiT tile layout.
            ag_in = ag_pool.tile([128, NJQ * NKH * 512], R, name="ag_in")
            ag_out = ag_pool.tile([4, 128, NJQ * NKH * 512], R, name="ag_out")

            # kiT per source rank m: [p, jl, kh*512+k]; head h == rank h's
            # tile (head h spans global j-tiles 4h..4h+3).
            kiT = [
                kiT_pool.tile([128, 4, LC], R, tag=f"kiT{m}", name=f"kiT{m}")
                for m in range(4)
            ]

            # The input ramp is paced by the DMA fabric warming up
            # (~60-150GB/s per queue over the first 25us), so the kproj
            # schedule is shaped around supply: all 8 accumulation chains
            # (4 jl x 2 kh, one PSUM bank each) run concurrently, walking
            # the contraction in dt-pairs, so each landing 256KB kT piece
            # feeds 8 matmuls (~120GB/s demand) and each landing 128KB wkb
            # column-quarter feeds 32. kT rides the sync queue; the wkb
            # quarters stream on the scalar queue in need order.
            with (
                nc.named_scope("kproj"),
                tc.tile_pool(name="kT", bufs=1) as kT_pool,
                tc.tile_pool(name="wkblk", bufs=1) as wkblk_pool,
                tc.tile_pool(name="kstg", bufs=8) as kstg_pool,
                tc.tile_pool(name="psA", bufs=8, space="PSUM") as psA_pool,
            ):
                wkb_sb = [
                    wkblk_pool.tile([128, D], R, tag=f"wkb{jl}", name=f"wkb{jl}")
                    for jl in range(NJQ)
                ]
                kT_sb = [
                    kT_pool.tile([128, 4, LC], R, tag=f"kTg{g}", name=f"kTg{g}")
                    for g in range(NG)
                ]
                for jl in range(NJQ):
                    nc.scalar.dma_start(
                        out=wkb_sb[jl][:, 0:512], in_=wk2q[jl][:, 0:512]
                    )
                for dt in range(ND):
                    g, dtl = dt // 4, dt % 4
                    nc.sync.dma_start(
                        out=kT_sb[g][:, dtl : dtl + 1, :],
                        in_=kT[g, :, dtl : dtl + 1, :],
                    )
                for qtr in range(1, 4):
                    for jl in range(NJQ):
                        nc.scalar.dma_start(
                            out=wkb_sb[jl][:, qtr * 512 : (qtr + 1) * 512],
                            in_=wk2q[jl][:, qtr * 512 : (qtr + 1) * 512],
                        )
                # stage-B inputs stream behind stage A's on both queues.
                qT_sb = [
                    qT_pool.tile([128, 4, LQC], R, tag=f"qTg{g}", name=f"qTg{g}")
                    for g in range(NG)
                ]
                for g in range(NG):
                    nc.scalar.dma_start(out=qT_sb[g][:], in_=qT[g])
                wg_sb = misc_pool.tile([128, ND * H], R, tag="wg", name="wg_sb")
                nc.scalar.dma_start(out=wg_sb[:], in_=wg2[:])
                eye_sb = misc_pool.tile([4, 4], RW, tag="eye", name="eye_sb")
                nc.scalar.dma_start(out=eye_sb[:], in_=eye4[:])
                wqp = []
                for t in range(5):
                    w = wqblk_pool.tile(
                        [128, 2, D], R, tag="wqb", name=f"wqp{t}", bufs=5
                    )
                    nc.sync.dma_start(out=w[:], in_=wq2[t])
                    wqp.append(w)
                # 8 concurrent chains, dt-pair round-robin across jl
                ps = {
                    (jl, kh): psA_pool.tile(
                        [128, 512], F32, tag="psA", name=f"psA{jl}_{kh}"
                    )
                    for jl in range(NJQ)
                    for kh in range(NKH)
                }
                for db in range(ND // 2):
                    for jl in range(NJQ):
                        for dt in (db * 2, db * 2 + 1):
                            g, dtl = dt // 4, dt % 4
                            for kh in range(NKH):
                                nc.tensor.matmul(
                                    ps[(jl, kh)][:],
                                    wkb_sb[jl][:, dt * 128 : (dt + 1) * 128],
                                    kT_sb[g][:, dtl : dtl + 1, kh * 512 : (kh + 1) * 512],
                                    start=(dt == 0),
                                    stop=(dt == ND - 1),
                                )
                for jl in range(NJQ):
                    for kh in range(NKH):
                        stg = kstg_pool.tile(
                            [128, 512], R, tag="kstg", name=f"kstg{jl}_{kh}"
                        )
                        nc.scalar.copy(stg[:], ps[(jl, kh)][:])
                        c = jl * NKH + kh
                        nc.sync.dma_start(
                            out=ag_in[:, c * 512 : (c + 1) * 512], in_=stg[:]
                        )

                # CC dispatch held behind the wq pair-4 prefetch: the gather
                # (dispatch + ~12us trigger + ~50us drain) still completes
                # ~50us before qk needs kiT.
                delay_t = misc_pool.tile([128, 8], R, tag="ccdel", name="ccdel")
                nc.gpsimd.tensor_copy(delay_t[:], wqp[4][:, 1:2, 0:8])
                nc.gpsimd.collective_compute(
                    "AllGather",
                    mybir.AluOpType.bypass,
                    replica_groups=REPLICA_GROUPS,
                    ins=[ag_in.opt()],
                    outs=[ag_out.opt()],
                )

            # ---------------- stage B: full i range ---------------------
            # psB/psw/pswt open after psA's 8 banks free up (the WAR wait
            # lands on jl0's staging copies, done ~0.6us after kproj) and
            # close together right before qk claims all 8 banks for psq.
            ps_stack = ExitStack()
            psB_pool = ps_stack.enter_context(
                tc.tile_pool(name="psB", bufs=2, space="PSUM")
            )
            psw_pool = ps_stack.enter_context(
                tc.tile_pool(name="psw", bufs=2, space="PSUM")
            )
            with (
                tc.tile_pool(name="qiT", bufs=1) as qiT_pool,
                tc.tile_pool(name="wsb", bufs=1) as w_pool,
                tc.tile_pool(name="sc", bufs=3) as sc_pool,
                tc.tile_pool(name="tm", bufs=4) as tm_pool,
            ):
                if True:
                    # gate vector: wT[h, i] halves, then 4x128 PE transposes
                    with nc.named_scope("wproj"):
                        w4 = dve_pool.tile([4, LQC], RW, tag="w4", name="w4")
                        for ih in range(2):
                            psw = psw_pool.tile([4, 512], F32, tag="psw", name=f"psw{ih}")
                            for dt in range(ND):
                                g, dtl = dt // 4, dt % 4
                                nc.tensor.matmul(
                                    psw[:],
                                    wg_sb[:, dt * H : (dt + 1) * H],
                                    qT_sb[g][:, dtl : dtl + 1, ih * 512 : (ih + 1) * 512],
                                    start=(dt == 0),
                                    stop=(dt == ND - 1),
                                )
                            nc.vector.tensor_copy(w4[:, ih * 512 : (ih + 1) * 512], psw[:])
                        w_sb = []
                        for it in range(NIT):
                            pswt = psw_pool.tile(
                                [128, 4], F32, tag="pswt", name=f"pswt{it}", bufs=4
                            )
                            nc.tensor.matmul(
                                pswt[:],
                                w4[:, it * 128 : (it + 1) * 128],
                                eye_sb[:],
                                start=True,
                                stop=True,
                            )
                            wt = w_pool.tile([128, 4], F32, tag=f"w{it}", name=f"w{it}")
                            nc.vector.tensor_copy(wt[:], pswt[:])
                            w_sb.append(wt)

                    # qiT = Wq-blocks.T @ qT (each weight block used once)
                    with nc.named_scope("qproj"):
                        qiT = []
                        for jt in range(NJ):
                            t, tl = jt // 2, jt % 2
                            if tl == 0 and 5 <= t + 2 <= 7:
                                w = wqblk_pool.tile(
                                    [128, 2, D], R, tag="wqb", name=f"wqp{t + 2}", bufs=5
                                )
                                nc.sync.dma_start(out=w[:], in_=wq2[t + 2])
                                wqp.append(w)
                            qi = qiT_pool.tile([128, LQC], R, tag=f"qiT{jt}", name=f"qiT{jt}")
                            for ih in range(2):
                                ps = psB_pool.tile([128, 512], F32, tag="psB", name=f"psB{jt}_{ih}")
                                for dt in range(ND):
                                    g, dtl = dt // 4, dt % 4
                                    nc.tensor.matmul(
                                        ps[:],
                                        wqp[t][:, tl : tl + 1, dt * 128 : (dt + 1) * 128],
                                        qT_sb[g][:, dtl : dtl + 1, ih * 512 : (ih + 1) * 512],
                                        start=(dt == 0),
                                        stop=(dt == ND - 1),
                                    )
                                nc.scalar.copy(qi[:, ih * 512 : (ih + 1) * 512], ps[:])
                            qiT.append(qi)

                    # full kiT comes back from the gather: one ~1MB DMA per
                    # source rank, landing in the [p, jl, kh*512+k] layout
                    # qk consumes directly.
                    for m in range(4):
                        nc.sync.dma_start(out=kiT[m][:], in_=ag_out[m : m + 1])

                ps_stack.close()  # free psA/psB/psw banks for qk's 8

                # qk + fused relu*w epilogue
                with (
                    nc.named_scope("qk"),
                    tc.tile_pool(name="psq", bufs=8, space="PSUM") as psq_pool,
                ):
                    for it in range(NIT):
                        sc = sc_pool.tile([128, LC], F32, tag="sc", name=f"sc{it}")
                        for kh in range(NKH):
                            last_unit = it == NIT - 1 and kh == NKH - 1
                            # the final unit runs in two 256-col chunks so
                            # the post-last-matmul epilogue + DMA is short
                            nch = 2 if last_unit else 1
                            cw = 512 // nch
                            for c in range(nch):
                                base = kh * 512 + c * cw
                                scs = sc[:, base : base + cw]
                                for h in range(H):
                                    psq = psq_pool.tile(
                                        [128, cw], F32, tag="psq",
                                        name=f"psq{it}_{kh}_{c}_{h}",
                                    )
                                    for j in range(4):
                                        nc.tensor.matmul(
                                            psq[:],
                                            qiT[h * 4 + j][:, it * 128 : (it + 1) * 128],
                                            kiT[h][:, j : j + 1, base : base + cw],
                                            start=(j == 0),
                                            stop=(j == 3),
                                        )
                                    if h == 0:
                                        nc.vector.tensor_scalar(
                                            out=scs,
                                            in0=psq[:],
                                            scalar1=0.0,
                                            scalar2=w_sb[it][:, 0:1],
                                            op0=mybir.AluOpType.max,
                                            op1=mybir.AluOpType.mult,
                                        )
                                    else:
                                        tm = tm_pool.tile(
                                            [128, cw], F32, tag="tm",
                                            name=f"tm{it}_{kh}_{c}_{h}",
                                        )
                                        nc.scalar.activation(
                                            tm[:], psq[:], mybir.ActivationFunctionType.Relu
                                        )
                                        nc.vector.scalar_tensor_tensor(
                                            out=scs,
                                            in0=tm[:],
                                            scalar=w_sb[it][:, h : h + 1],
                                            in1=scs,
                                            op0=mybir.AluOpType.mult,
                                            op1=mybir.AluOpType.add,
                                        )
                                dst = scores[
                                    it * 128 : (it + 1) * 128, base : base + cw
                                ]
                                if last_unit and c % 2 == 1:
                                    # the issuing engine (scalar) just ran
                                    # this chunk's relu; the issue slots in
                                    # while the DVE finishes the stt
                                    nc.scalar.dma_start(out=dst, in_=scs)
                                else:
                                    nc.sync.dma_start(out=dst, in_=scs)
    nc.finalize()
    return nc


def _get_program():
    if "nc" not in _CACHE:
        _CACHE["nc"] = _build()
    return _CACHE["nc"]


def _tile_weight(w):
    # [j, d] nn.Linear weight -> [jt, p, dt*128+jcol] blocks where
    # block[jt][p, dt*128+j] = W.T[dt*128+p, jt*128+j]
    a = w.reshape(NJ, 128, ND, 128)  # [jt, j, dt, p]
    return np.ascontiguousarray(a.transpose(0, 3, 2, 1)).reshape(NJ, 128, D)


def _group_dmajor(xT):
    # [D, cols] d-major -> [g, p, dtl, cols] with d = g*512 + dtl*128 + p
    cols = xT.shape[1]
    a = xT.reshape(NG, 4, 128, cols)  # [g, dtl, p, cols]
    return np.ascontiguousarray(a.transpose(0, 2, 1, 3))


def _shard_inputs(q, k_compressed, Wq, Wk, Wg):
    ndt = np.float16
    wq_blocks = _tile_weight(np.asarray(Wq, dtype=np.float32)).astype(ndt)
    # pair j-tiles (2t, 2t+1): [t, p, tl, d]
    wq2 = np.ascontiguousarray(
        wq_blocks.reshape(NJ // 2, 2, 128, D).transpose(0, 2, 1, 3)
    )
    wk2 = _tile_weight(np.asarray(Wk, dtype=np.float32)).astype(ndt)
    # wg2[p, dt*4+h] = Wg.T[dt*128+p, h]
    g = np.asarray(Wg, dtype=np.float32).reshape(H, ND, 128)  # [h, dt, p]
    wg2 = np.ascontiguousarray(g.transpose(2, 1, 0)).reshape(128, ND * H).astype(ndt)
    eye = np.eye(4, dtype=np.float32)

    in_maps = []
    for c in range(N_CORES):
        b = c // (N_CORES // B)
        cq = c % (N_CORES // B)
        i0 = cq * LQC
        qTc = _group_dmajor(
            np.ascontiguousarray(
                np.asarray(q[b, i0 : i0 + LQC, :], dtype=np.float32).T
            ).astype(ndt)
        )
        kTc = _group_dmajor(
            np.ascontiguousarray(
                np.asarray(k_compressed[b], dtype=np.float32).T
            ).astype(ndt)
        )
        wk2q = np.ascontiguousarray(wk2[cq * NJQ : (cq + 1) * NJQ])
        in_maps.append(
            {"qT": qTc, "kT": kTc, "wq2": wq2, "wk2q": wk2q, "wg2": wg2, "eye4": eye}
        )
    return in_maps


def _run(inputs, trace=False, **kw):
    nc = _get_program()
    in_maps = _shard_inputs(**inputs)
    res = run_bass_kernel_spmd(nc, in_maps, list(range(N_CORES)), trace=trace, **kw)
    out = np.empty((B, LQ, LC), dtype=np.float32)
    for c in range(N_CORES):
        b = c // (N_CORES // B)
        i0 = (c % (N_CORES // B)) * LQC
        out[b, i0 : i0 + LQC, :] = res.results[c]["scores"]
    return out, res


def kernel(**inputs) -> np.ndarray:
    out, _ = _run(inputs)
    return out
